# revision 33
# baseline (speedup 1.0000x reference)
"""MoE-with-DeepGEMM kernel for 8 Trainium2 NeuronCores.

Problem: M=4096 tokens, D=2048 in-dim, H=2048 out-dim, E=8 experts.
    gate = softmax(x @ gate_w.T + gate_b)            # [M, E], fp32
    y    = (q8(x) @ q8(expert_w[e]).T) -> bf16       # [E, M, H]
    out  = sum_e gate[:, e, None] * y[e].astype(f32) # [M, H]

Strategy: data-parallel over tokens (M). Each of the 8 cores gets
M/8 = 512 tokens, all 8 experts' weights, and computes its output slice
independently - no collectives; the host concatenates the slices.

The PE floor is 1024 DoubleRow fp8 matmuls (N=512) x 216 ns = 221 us
plus 3.5 us of fp16 gating matmuls; everything else is scheduled to hide
under that:
  - all inputs are pre-arranged on the HOST into the exact SBUF layout
    [128 partitions, subtile, col], so every DMA is a wide contiguous
    copy (max line size, cheap descriptors),
  - the whole startup stream rides ONE queue (Sync HWDGE - fastest
    first byte, FIFO) in exact consumption order: xq s-slices just
    ahead of the w0 pieces they multiply against, then xf/gwt, then
    w1..w7 whole (one piece per expert, double-buffered two ahead).
    HBM is shared by all 8 cores, so aggregate delivery (~280-350 GB/s)
    is the constraint - a second queue just reorders completions and
    delays the critical piece (measured, repeatedly),
  - expert 0 computes in two h-half phases (4 mc x 2 h-chunks = 8 PSUM
    banks, k-major), so phase 1 needs only xq + half of w0 (2.75 MB)
    and even a HAM-cold PE never starves; no warm-up matmuls - real
    matmuls start the moment the first two pieces land (blocking the
    queue behind junk warm-ups measured strictly worse),
  - the per-expert combine (acc += gate * y) reads PSUM directly on DVE
    (skipping the reference's bf16 round-trip costs ~1e-3 rel err, well
    inside the 2e-2 budget) - no y tiles, fewer semaphores,
  - expert 0 runs before gating is known: its PSUM is copied raw into
    acc (ACT engine), then scaled in place by gate[:,0] after softmax.
    Gating runs at the e0/e1 boundary; ACT (idle there) moves the
    logits PSUM->SBUF so the PE transposes don't wait on DVE,
  - the last expert runs hc-major within each mc so each PSUM bank
    finishes its 8-matmul k-loop early, letting combine + output DMA
    drain behind the PE; the final chunk is split in two 128 KB pieces.

(Trace note: the NTFF profile drops ~1 instruction record per ~100, so
apparent periodic 432 ns "stalls" every ~10.8 us are missing-MATMUL
artifacts, not real gaps - the steady region runs at the 216 ns floor.)
"""

import numpy as np
import ml_dtypes

import concourse.bacc as bacc
import concourse.bass as bass
import concourse.mybir as mybir
import concourse.tile as tile
from concourse import masks
from concourse.bass_utils import run_bass_kernel_spmd

M, D, H, E = 4096, 2048, 2048, 8
NCORES = 8
MS = M // NCORES          # tokens per core (512)
MC = MS // 128            # m-chunks of 128 partitions (4)
DS = D // 128             # d-subtiles of 128 (16)
KP = DS // 2              # DoubleRow d-pairs of 256 (8)
NH = 512                  # h columns per matmul (one PSUM bank of f32)
HC = H // NH              # h-chunks (4)

_NC = None


def _build_program() -> bass.Bass:
    dt = mybir.dt
    nc = bacc.Bacc(None, target_bir_lowering=False)

    # Host pre-arranges everything into [128, subtile, col] SBUF order.
    xq = nc.dram_tensor("xq", [128, DS * MS], dt.float8e4, kind="ExternalInput")
    xf = nc.dram_tensor("xf", [128, DS * MS], dt.float16, kind="ExternalInput")
    wq = nc.dram_tensor("wq", [E * 128, DS * H], dt.float8e4, kind="ExternalInput")
    gwt = nc.dram_tensor("gwt", [128, DS * E], dt.float16, kind="ExternalInput")
    gb = nc.dram_tensor("gb", [E, 1], dt.float32, kind="ExternalInput")
    out = nc.dram_tensor("out", [MS, H], dt.float32, kind="ExternalOutput")

    with tile.TileContext(nc) as tc, \
            tc.tile_pool(name="const", bufs=1) as constp, \
            tc.tile_pool(name="wpool", bufs=3) as wpool, \
            tc.tile_pool(name="small", bufs=8) as small, \
            tc.tile_pool(name="ps", bufs=8, space="PSUM") as psp:

        # Persistent SBUF tensors. Contraction index d = s*128 + p.
        xq_sb = constp.tile([128, DS, MS], dt.float8e4, tag="xq")
        xf_sb = constp.tile([128, DS, MS], dt.float16, tag="xf")
        gwt_sb = constp.tile([128, DS, E], dt.float16, tag="gwt")
        gb_sb = constp.tile([E, 1], dt.float32, tag="gb")
        id8_sb = constp.tile([E, E], dt.float32, tag="id8")
        gate_sb = constp.tile([128, MC * E], dt.float32, tag="gate")
        lg_sb = constp.tile([E, MS], dt.float32, tag="lg")
        acc_sb = constp.tile([128, MC * H], dt.float32, tag="acc")

        nc.gpsimd.dma_start(gb_sb[:], gb[:, :])
        masks.make_identity(nc, id8_sb[:])

        def dma_xq(lo, hi):
            return nc.sync.dma_start(
                xq_sb[:, lo:hi, :],
                xq[:, lo * MS:hi * MS].rearrange("p (s m) -> p s m", m=MS))

        def emit_gating():
            ps_gt = psp.tile([E, MS], dt.float32, tag="ps", name="ps_gt")
            for s in range(DS):
                nc.tensor.matmul(
                    ps_gt[:],
                    lhsT=gwt_sb[:, s:s + 1, :],
                    rhs=xf_sb[:, s:s + 1, :],
                    start=(s == 0),
                    stop=(s == DS - 1),
                )
            # ACT does the PSUM->SBUF logits move (+ bias): it's idle
            # here, while DVE on the critical path cost the transposes
            # an extra ~1.3us.
            nc.scalar.add(lg_sb[:], ps_gt[:], gb_sb[:])

        def emit_softmax():
            for mc in range(MC):
                pst = psp.tile([128, E], dt.float32, tag="ps", name=f"ps_t{mc}")
                nc.tensor.transpose(
                    pst[:], lg_sb[:, mc * 128:(mc + 1) * 128], id8_sb[:]
                )
                mx = small.tile([128, 1], dt.float32, tag="sm1")
                nc.vector.tensor_reduce(
                    mx[:], pst[:], mybir.AxisListType.X, mybir.AluOpType.max
                )
                nmx = small.tile([128, 1], dt.float32, tag="sm1")
                nc.vector.tensor_scalar_mul(nmx[:], mx[:], -1.0)
                ex = small.tile([128, E], dt.float32, tag="sm")
                ssum = small.tile([128, 1], dt.float32, tag="sm1")
                nc.scalar.activation(
                    ex[:], pst[:], mybir.ActivationFunctionType.Exp,
                    bias=nmx[:], scale=1.0, accum_out=ssum[:],
                )
                rcp = small.tile([128, 1], dt.float32, tag="sm1")
                nc.vector.reciprocal(rcp[:], ssum[:])
                nc.vector.tensor_scalar_mul(
                    gate_sb[:, mc * E:(mc + 1) * E], ex[:], rcp[:]
                )

        # ---- Main GEMM + weighted combine ----
        for e in range(E):
            w_sb = wpool.tile([128, DS, H], dt.float8e4, tag="w")
            if e == 0:
                # Startup feed: ONE queue (Sync - fast start, FIFO), in
                # exact consumption order: xq s-slices just ahead of the
                # w0 k-pair slices they multiply against, then the gating
                # inputs, then w1 (emitted next iteration). HBM is shared
                # by all 8 cores, so aggregate delivery (~280-350 GB/s)
                # is the constraint - a second queue just reorders
                # completions and delays the critical piece.
                # Expert 0 computes in two h-half phases (all 4 mc x 2
                # h-chunks = 8 banks), so phase 1 only needs xq + the h0
                # half of w0 (2.75 MB) - the feed keeps ahead of even a
                # cold-clock PE. w0 streams as 16 quarter-pieces (256 KB,
                # k-pair x h-half) in exact consumption order.
                def w0_piece(kp, lo, hi):
                    nc.sync.dma_start(
                        w_sb[:, 2 * kp:2 * kp + 2, lo:hi],
                        wq[0:128, kp * 2 * H:(kp + 1) * 2 * H].rearrange(
                            "p (s h) -> p s h", h=H)[:, :, lo:hi],
                    )
                dma_xq(0, 2)
                w0_piece(0, 0, 1024)
                dma_xq(2, 4)
                w0_piece(1, 0, 1024)
                dma_xq(4, 8)
                w0_piece(2, 0, 1024)
                w0_piece(3, 0, 1024)
                dma_xq(8, 12)
                w0_piece(4, 0, 1024)
                w0_piece(5, 0, 1024)
                dma_xq(12, 16)
                w0_piece(6, 0, 1024)
                w0_piece(7, 0, 1024)
                for kp in range(KP):
                    w0_piece(kp, 1024, 2048)
                for j in range(2):
                    nc.sync.dma_start(
                        xf_sb[:, j * 8:(j + 1) * 8, :],
                        xf[:, j * 8 * MS:(j + 1) * 8 * MS].rearrange(
                            "p (s m) -> p s m", m=MS),
                    )
                nc.sync.dma_start(
                    gwt_sb[:], gwt[:, :].rearrange("p (s e) -> p s e", e=E))
            else:
                # Later experts load whole (fewer sem waits, which
                # otherwise split into extra LDWEIGHTS slots).
                nc.sync.dma_start(
                    w_sb[:],
                    wq[e * 128:(e + 1) * 128, :].rearrange(
                        "p (s h) -> p s h", h=H),
                )
            if e == 0:
                # Two h-half phases, k-major across all 4 mc chunks. Raw
                # PSUM is copied to acc on ACT; the gate scale is applied
                # in place after softmax.
                for hp in range(2):
                    pss = {
                        mc: [
                            psp.tile([128, NH], dt.float32, tag="ps",
                                     name=f"ps0_{hp}_{mc}_{i}")
                            for i in range(2)
                        ]
                        for mc in range(MC)
                    }
                    for k in range(KP):
                        for mc in range(MC):
                            lhsT = xq_sb[:, 2 * k:2 * k + 2,
                                         mc * 128:(mc + 1) * 128]
                            for hh in range(2):
                                hc = 2 * hp + hh
                                nc.tensor.matmul(
                                    pss[mc][hh][:],
                                    lhsT=lhsT,
                                    rhs=w_sb[:, 2 * k:2 * k + 2,
                                             hc * NH:(hc + 1) * NH],
                                    start=(k == 0),
                                    stop=(k == KP - 1),
                                    perf_mode=mybir.MatmulPerfMode.DoubleRow,
                                )
                    for mc in range(MC):
                        for hh in range(2):
                            hc = 2 * hp + hh
                            nc.scalar.copy(
                                acc_sb[:, mc * H + hc * NH:
                                       mc * H + (hc + 1) * NH],
                                pss[mc][hh][:],
                            )
                # Gating at the e0/e1 boundary: mc 2's PSUM banks freed
                # early (sequential phases) so the softmax transposes get
                # slots without stalling the PE.
                emit_gating()
                emit_softmax()
                for mc in range(MC):
                    g0_ap = gate_sb[:, mc * E:mc * E + 1]
                    for hc in range(HC):
                        a_ap = acc_sb[:, mc * H + hc * NH:
                                      mc * H + (hc + 1) * NH]
                        nc.scalar.mul(a_ap, a_ap, g0_ap)
                continue

            last = e == E - 1
            for mc in range(MC):
                msl = slice(mc * 128, (mc + 1) * 128)
                pss = [
                    psp.tile([128, NH], dt.float32, tag="ps",
                             name=f"ps_{e}_{mc}_{i}")
                    for i in range(HC)
                ]
                g_ap = gate_sb[:, mc * E + e:mc * E + e + 1]
                if last:
                    # hc-major: each bank completes its 8-matmul k-loop
                    # before the next starts, so combine + out DMA drain
                    # behind the PE instead of after it. The very last
                    # chunk is split in two so the post-matmul tail is a
                    # 256-col combine plus a 128 KB DMA.
                    for hc in range(HC):
                        for k in range(KP):
                            nc.tensor.matmul(
                                pss[hc][:],
                                lhsT=xq_sb[:, 2 * k:2 * k + 2, msl],
                                rhs=w_sb[:, 2 * k:2 * k + 2,
                                         hc * NH:(hc + 1) * NH],
                                start=(k == 0),
                                stop=(k == KP - 1),
                                perf_mode=mybir.MatmulPerfMode.DoubleRow,
                            )
                        final = mc == MC - 1 and hc == HC - 1
                        for q in range(2) if final else range(1):
                            csl = slice(hc * NH + (NH // 2) * q,
                                        hc * NH + (NH // 2) * (q + 1)
                                        ) if final else slice(
                                            hc * NH, (hc + 1) * NH)
                            a_ap = acc_sb[:, mc * H + csl.start:
                                          mc * H + csl.stop]
                            p_ap = pss[hc][:, csl.start - hc * NH:
                                           csl.stop - hc * NH]
                            nc.vector.scalar_tensor_tensor(
                                a_ap, p_ap, g_ap, a_ap,
                                op0=mybir.AluOpType.mult,
                                op1=mybir.AluOpType.add,
                            )
                            eng = nc.scalar if (mc * HC + hc) % 2 == 0 \
                                else nc.sync
                            eng.dma_start(
                                out[mc * 128:(mc + 1) * 128, csl], a_ap)
                else:
                    for k in range(KP):
                        lhsT = xq_sb[:, 2 * k:2 * k + 2, msl]
                        for hc in range(HC):
                            nc.tensor.matmul(
                                pss[hc][:],
                                lhsT=lhsT,
                                rhs=w_sb[:, 2 * k:2 * k + 2,
                                         hc * NH:(hc + 1) * NH],
                                start=(k == 0),
                                stop=(k == KP - 1),
                                perf_mode=mybir.MatmulPerfMode.DoubleRow,
                            )
                    for hc in range(HC):
                        a_ap = acc_sb[:, mc * H + hc * NH:
                                      mc * H + (hc + 1) * NH]
                        nc.vector.scalar_tensor_tensor(
                            a_ap, pss[hc][:], g_ap, a_ap,
                            op0=mybir.AluOpType.mult,
                            op1=mybir.AluOpType.add,
                        )

    nc.compile()
    return nc


def _get_nc() -> bass.Bass:
    global _NC
    if _NC is None:
        _NC = _build_program()
    return _NC


def _sbuf_order(a, cols):
    """[D, cols] -> [128, DS_a * cols] contiguous in (p, s, col) order."""
    d = a.shape[0]
    return np.ascontiguousarray(
        a.reshape(d // 128, 128, cols).transpose(1, 0, 2)
    ).reshape(128, (d // 128) * cols)


def _prep_in_maps(x, gate_w, gate_b, expert_w):
    f8fn = ml_dtypes.float8_e4m3fn
    f8trn = ml_dtypes.float8_e4m3  # same bits as e4m3fn for |v| <= 240

    x = np.asarray(x, dtype=np.float32)
    gate_w = np.asarray(gate_w, dtype=np.float32)
    gate_b = np.asarray(gate_b, dtype=np.float32)
    expert_w = np.asarray(expert_w, dtype=np.float32)

    xT = np.ascontiguousarray(x.T)                       # [D, M] f32
    xT_f16 = xT.astype(np.float16)                       # gating copy
    xqT = xT.astype(f8fn).view(f8trn)                    # [D, M] fp8
    # expert_w [E, H, D] -> per-expert w^T [D, H], quantized, then into
    # SBUF order [E*128, DS*H].
    wqT = np.ascontiguousarray(
        expert_w.transpose(0, 2, 1)
    ).astype(f8fn).view(f8trn)                           # [E, D, H]
    wq_l = np.ascontiguousarray(
        wqT.reshape(E, DS, 128, H).transpose(0, 2, 1, 3)
    ).reshape(E * 128, DS * H)
    gwt_l = _sbuf_order(
        np.ascontiguousarray(gate_w.T).astype(np.float16), E)
    gbb = np.ascontiguousarray(gate_b.reshape(E, 1))

    in_maps = []
    for c in range(NCORES):
        csl = slice(c * MS, (c + 1) * MS)
        in_maps.append({
            "xq": _sbuf_order(np.ascontiguousarray(xqT[:, csl]), MS),
            "xf": _sbuf_order(np.ascontiguousarray(xT_f16[:, csl]), MS),
            "wq": wq_l,
            "gwt": gwt_l,
            "gb": gbb,
        })
    return in_maps


def kernel(x, gate_w, gate_b, expert_w, _trace=False, _trace_kwargs=None):
    nc = _get_nc()
    in_maps = _prep_in_maps(x, gate_w, gate_b, expert_w)
    kw = {}
    if _trace:
        kw["trace"] = True
        kw.update(_trace_kwargs or {})
    res = run_bass_kernel_spmd(nc, in_maps, core_ids=list(range(NCORES)), **kw)
    outp = np.concatenate(
        [np.asarray(res.results[c]["out"]) for c in range(NCORES)], axis=0
    )
    if _trace:
        return outp, res
    return outp
